# revision 33
# baseline (speedup 1.0000x reference)
"""Trainium2 Bass kernel for nn_AttentionCircuit (mixture-routed attention).

Wire-transfer-minimal SPMD design (8 cores, single program). The axon tunnel
(~35 MB/s) dominates wall-clock, so every staged tensor is fp16 and sharded
with no cross-core replication of large data:

  - project: token-sharded (core c: global tokens [512c, 512c+512)); the
    F neuron banks are staged D-sharded (128 rows/core) and AllGathered
    on-chip; h vectors AllGathered (tiny).
  - restore: nr-sharded (core c: neurons [4c, 4c+4) = 256 of 2048 nr rows);
    partial Y^T accumulated in f32 and ReduceScattered over the D axis so
    each core ends with its 128-dim head slice (2 heads) for all tokens.
  - attention + W_O: head-sharded (2 heads x 2 batches per core); W_O
    partials ReduceScattered over tokens on-chip; each core returns a
    [512, 1024] fp16 slice of the final output.

On-chip numerics: fp16 matmuls with f32 PSUM accumulation everywhere except
the attention score matmul, which runs fp32r on f32 Q/K (softmax scores here
reach |s|~1900, so Q/K are never rounded below f32 after the restore).
"""
import sys
sys.path.insert(0, "/opt/trn_rl_repo")
import numpy as np
from contextlib import ExitStack

import concourse.bacc as bacc
import concourse.mybir as mybir
from concourse import tile
from concourse.masks import make_identity, make_upper_triangular
from concourse import bass2jax as _b2j
from concourse.bass_utils import (run_bass_kernel_spmd as _stock_run_spmd,
                                  BassKernelResults as _BKR)
import jax
from jax.experimental.shard_map import shard_map
from jax.sharding import Mesh, PartitionSpec, NamedSharding

B, S, D, R, H, DH, N = 2, 2048, 1024, 64, 16, 64, 32
NR = N * R            # 2048
T = B * S             # 4096 global tokens
P = 128
TOK = 512             # tokens per core (project shard)
NL = 4                # neurons per core (restore shard)
KRL = NL * R          # 256 local nr rows
DL = 128              # local d slice (2 heads) for attention/W_O
GROUPS = [[0, 1, 2, 3, 4, 5, 6, 7]]
F32 = mybir.dt.float32
F16 = mybir.dt.float16
F32R = mybir.dt.float32r
U16 = mybir.dt.uint16
SHR = mybir.AluOpType.logical_shift_right
SHL = mybir.AluOpType.logical_shift_left
AND = mybir.AluOpType.bitwise_and
ORR = mybir.AluOpType.bitwise_or
MULT = mybir.AluOpType.mult
ADD = mybir.AluOpType.add
AXX = mybir.AxisListType.X
EXP = mybir.ActivationFunctionType.Exp

_CACHED = {}
_EXEC_CACHE = {}


def _get_exec(nc, n_cores):
    """Build (once) the jitted SPMD executable for `nc`, mirroring the axon
    branch of concourse.bass_utils.run_bass_kernel_spmd, with two host-path
    savings: the jit closure is cached across calls, and the zero-filled
    ExternalOutput staging buffers live on-device instead of being shipped
    over the tunnel on every call. Inputs still transfer fully per call."""
    key = (id(nc), n_cores)
    if key in _EXEC_CACHE:
        return _EXEC_CACHE[key]
    _b2j.install_neuronx_cc_hook()
    partition_name = (nc.partition_id_tensor.name
                      if nc.partition_id_tensor else None)
    in_names, out_names, out_avals, zero_outs = [], [], [], []
    for alloc in nc.m.functions[0].allocations:
        if not isinstance(alloc, mybir.MemoryLocationSet):
            continue
        name = alloc.memorylocations[0].name
        if alloc.kind == "ExternalInput":
            if name != partition_name:
                in_names.append(name)
        elif alloc.kind == "ExternalOutput":
            shape = tuple(alloc.tensor_shape)
            dtype = mybir.dt.np(alloc.dtype)
            out_names.append(name)
            out_avals.append(jax.core.ShapedArray(shape, dtype))
            zero_outs.append(np.zeros(shape, dtype))
    n_params = len(in_names)
    bind_names = list(in_names) + list(out_names)
    if partition_name is not None:
        bind_names.append(partition_name)

    def _body(*args):
        operands = list(args)
        if partition_name is not None:
            operands.append(_b2j.partition_id_tensor())
        outs = _b2j._bass_exec_p.bind(
            *operands,
            out_avals=tuple(out_avals),
            in_names=tuple(bind_names),
            out_names=tuple(out_names),
            lowering_input_output_aliases=(),
            sim_require_finite=True,
            sim_require_nnan=True,
            nc=nc,
        )
        return tuple(outs)

    devices = jax.devices()[:n_cores]
    mesh = Mesh(np.asarray(devices), ("core",))
    n_outs = len(out_names)
    fn = jax.jit(
        shard_map(_body, mesh=mesh,
                  in_specs=(PartitionSpec("core"),) * (n_params + n_outs),
                  out_specs=(PartitionSpec("core"),) * n_outs,
                  check_rep=False),
        keep_unused=True)
    sh = NamedSharding(mesh, PartitionSpec("core"))
    zeros_dev = [jax.device_put(
                     np.zeros((n_cores * z.shape[0], *z.shape[1:]), z.dtype),
                     sh)
                 for z in zero_outs]
    info = (fn, in_names, out_names, out_avals, zeros_dev)
    _EXEC_CACHE[key] = info
    return info


def run_bass_kernel_spmd(nc, in_maps, core_ids, **kwargs):
    tr = kwargs.pop("trace", False)
    te = kwargs.pop("trace_events", False)
    if tr or te or kwargs:
        return _stock_run_spmd(nc, in_maps, core_ids, trace=tr,
                               trace_events=te, **kwargs)
    n = len(core_ids)
    fn, in_names, out_names, out_avals, zeros_dev = _get_exec(nc, n)

    def _concat(k):
        arrs = [np.asarray(in_maps[c][k]) for c in range(n)]
        # if the per-core maps are row-views of one prebuilt global array
        # (kernel() builds them that way), skip the 25MB concat memcpy
        g = _CACHED.get("global_" + k)
        if g is not None and g.shape[0] % n == 0:
            rows = g.shape[0] // n
            if all(a.shape == (rows, *g.shape[1:]) and a.dtype == g.dtype and
                   a.__array_interface__["data"][0] ==
                   g[rows * c:].__array_interface__["data"][0]
                   for c, a in enumerate(arrs)):
                return g
        return np.concatenate(arrs, axis=0)

    concat_in = [_concat(k) for k in in_names]
    out_arrs = fn(*concat_in, *zeros_dev)
    # fetch the 8 device shards concurrently — the per-shard tunnel RTT
    # otherwise serializes (~40ms each)
    from concurrent.futures import ThreadPoolExecutor
    with ThreadPoolExecutor(8) as ex:
        def _fetch(a):
            shards = sorted(a.addressable_shards, key=lambda s: s.index[0].start or 0)
            parts = list(ex.map(lambda s: np.asarray(s.data), shards))
            return np.concatenate(parts, axis=0)
        fetched = [_fetch(a).reshape(n, *out_avals[i].shape)
                   for i, a in enumerate(out_arrs)]
    results = [
        {name: fetched[i][c] for i, name in enumerate(out_names)}
        for c in range(n)
    ]
    return _BKR(results=results, instructions_and_trace=None,
                profile_json=None, exec_time_ns=None)


def _r(ap):
    return ap.bitcast(F32R)


def _unpack12(nc, pool, wv, out_t, G):
    """Unpack [P, G, 3] uint16 words (4 x 12-bit codes per 3 words) into the
    fp16 tile out_t ([P, 4G] values; value bits = code << 4)."""
    hv = out_t[:].bitcast(U16).rearrange("p (g f) -> p g f", f=4)
    w0, w1, w2 = wv[:, :, 0], wv[:, :, 1], wv[:, :, 2]
    nc.vector.tensor_scalar(out=hv[:, :, 0], in0=w0, scalar1=0xFFF,
                            scalar2=4, op0=AND, op1=SHL)
    a1 = pool.tile([P, G], U16, tag="upk", name="upk")
    nc.vector.tensor_scalar(out=a1[:], in0=w0, scalar1=12, scalar2=4,
                            op0=SHR, op1=SHL)
    a2 = pool.tile([P, G], U16, tag="upk", name="upk")
    nc.vector.tensor_scalar(out=a2[:], in0=w1, scalar1=0xFF, scalar2=8,
                            op0=AND, op1=SHL)
    nc.vector.tensor_tensor(out=hv[:, :, 1], in0=a1[:], in1=a2[:], op=ORR)
    b1 = pool.tile([P, G], U16, tag="upk", name="upk")
    nc.vector.tensor_scalar(out=b1[:], in0=w1, scalar1=8, scalar2=4,
                            op0=SHR, op1=SHL)
    b2 = pool.tile([P, G], U16, tag="upk", name="upk")
    nc.vector.tensor_scalar(out=b2[:], in0=w2, scalar1=0xF, scalar2=12,
                            op0=AND, op1=SHL)
    nc.vector.tensor_tensor(out=hv[:, :, 2], in0=b1[:], in1=b2[:], op=ORR)
    nc.vector.tensor_scalar(out=hv[:, :, 3], in0=w2, scalar1=0xFFF0,
                            scalar2=None, op0=AND)


def build():
    nc = bacc.Bacc(None, target_bir_lowering=False)

    def dp(name, shape, dt=F16, out=False):
        return nc.declare_dram_parameter(name, list(shape), dt, isOutput=out)

    # All per-core inputs packed into ONE param ([400, 4096] u16 = 3.13MB)
    # — a single wire transfer per call instead of 11. Row map (4096-elem
    # u16 rows): 0-127 xT(f16), 128-191 Fqk(f16), 192-239 Fv(12-bit packed),
    # 240-251 fw q/k/v(f16), 252-263 rwT q/k/v(f16), 264-327 Rqk(f16),
    # 328-375 Rv(12-bit packed), 376-399 WOT(12-bit packed). 12-bit = fp16
    # RTN-rounded at mantissa bit 4, 4 codes per 3 uint16 words.
    blob_d = dp("blob", [400, 4096], dt=U16)
    xT_d = blob_d[0:128, :].bitcast(F16) \
        .rearrange("p (a t) -> (p a) t", a=8)                           # [D, TOK]
    fw_d = [blob_d[240 + 4 * i:244 + 4 * i, :].bitcast(F16)
            .rearrange("p (a n) -> (p a) n", a=128) for i in range(3)]  # [TOK, N]
    rwT_d = [blob_d[252 + 4 * i:256 + 4 * i, :].bitcast(F16)
             for i in range(3)]                                         # [NL, T]
    Rqk_d = blob_d[264:328, :].bitcast(F16) \
        .rearrange("p (a d) -> (p a) d", a=4)                           # [KRL, D]
    Rvp_d = blob_d[328:376, :].rearrange("a t -> (a t)") \
        .rearrange("(p t) -> p t", t=768)                               # [KRL, 768] u16
    WOTp_d = blob_d[376:400, :].rearrange("a t -> (a t)") \
        .rearrange("(p t) -> p t", t=768)                               # [DL, 768] u16
    # output: 12-bit floats (fp16 >> 4), 4 codes packed per 3 uint16 words
    out_d = dp("outp", [TOK, 3 * D // 4], dt=U16, out=True)

    tog = [0]

    def cp(out, in_):
        tog[0] ^= 1
        if tog[0]:
            nc.scalar.copy(out, in_)
        else:
            nc.vector.tensor_copy(out, in_)

    with ExitStack() as ctx:
        tc = ctx.enter_context(tile.TileContext(nc))
        const = ctx.enter_context(tc.tile_pool(name="const", bufs=1))
        ident32 = const.tile([P, P], F32, name="id32")
        make_identity(nc, ident32[:])
        ident16 = const.tile([P, P], F16, name="id16")
        make_identity(nc, ident16[:])
        maskU = const.tile([P, P], F32, name="maskU")
        make_upper_triangular(nc, maskU[:], val=1.0, diag=False)

        dram = ctx.enter_context(tc.tile_pool(name="dram", bufs=1, space="DRAM"))
        FG = dram.tile([8 * 112, 4096], U16, name="FG")          # gathered F
        hT_stack = dram.tile([3 * R, TOK], F16, name="hTstack")
        hT_gath = dram.tile([8 * 3 * R, TOK], F16, name="hTgath")
        yt_part = [dram.tile([D, T], F32, name=f"ytp{i}") for i in range(3)]
        yt_full = [dram.tile([DL, T], F32, name=f"ytf{i}") for i in range(3)]
        out_part = dram.tile([T, D], F32, name="outpart")
        out_rs = dram.tile([TOK, D], F32, name="outrs")

        # ---- AllGather the D-sharded F banks (0.9MB in, 7MB out) ----
        # (collectives cannot read IO tensors directly: bounce via SBUF)
        Fstage = dram.tile([112, 4096], U16, name="Fstage")
        with tc.tile_pool(name="fbounce", bufs=1) as fb:
            t = fb.tile([112, 4096], U16, tag="fb", name="fb")
            nc.sync.dma_start(out=t[:], in_=blob_d[128:240, :])
            nc.sync.dma_start(out=Fstage[:], in_=t[:])
        nc.gpsimd.collective_compute(
            "AllGather", mybir.AluOpType.bypass, replica_groups=GROUPS,
            ins=[Fstage[:].opt()], outs=[FG[:].opt()])

        # ---------------- Phase A: project (token-sharded) ----------------
        hT_pool = ctx.enter_context(tc.tile_pool(name="hTp", bufs=3))
        with tc.tile_pool(name="xF", bufs=72) as xF, \
             tc.tile_pool(name="fwp", bufs=6) as fwp, \
             tc.tile_pool(name="tmpp", bufs=3) as tmpp, \
             tc.tile_pool(name="hp", bufs=12) as hp, \
             tc.tile_pool(name="psA", bufs=6, space="PSUM") as psA, \
             tc.tile_pool(name="psH", bufs=2, space="PSUM") as psH:
            xT_sb = []
            for dc in range(8):
                t = xF.tile([P, TOK], F16, tag="xT", name="xT")
                nc.sync.dma_start(out=t[:], in_=xT_d[dc * P:(dc + 1) * P, :])
                xT_sb.append(t)
            fw_sb = []
            for ti in range(3):
                t = fwp.tile([P, 4 * N], F16, tag="fw", name="fw")
                nc.sync.dma_start(
                    out=t[:].rearrange("p (u n) -> p u n", u=4),
                    in_=fw_d[ti].rearrange("(u p) n -> p u n", p=P))
                t32 = fwp.tile([P, 4 * N], F32, tag="fw32", name="fw32")
                nc.vector.tensor_copy(t32[:], t[:])
                fw_sb.append(t32)
            F_sb = {}  # (bank, dc, ns) -> [P, 512]
            with tc.tile_pool(name="fpk", bufs=2) as fpk, \
                 tc.tile_pool(name="upkA", bufs=8) as upkA:
                for dc in range(8):
                    base = 112 * dc
                    fqk_v = FG[base:base + 64, :].bitcast(F16) \
                        .rearrange("p (a t) -> (p a) t", a=2)
                    for ns in range(4):
                        t = xF.tile([P, 512], F16, tag="F", name="F")
                        nc.sync.dma_start(
                            out=t[:], in_=fqk_v[:, ns * 512:(ns + 1) * 512])
                        F_sb[(0, dc, ns)] = t
                    pt = fpk.tile([P, 1536], U16, tag="Fp", name="Fp")
                    nc.sync.dma_start(
                        out=pt[:],
                        in_=FG[base + 64:base + 112, :]
                            .rearrange("a t -> (a t)")
                            .rearrange("(p t) -> p t", t=1536))
                    wv = pt[:].rearrange("p (g f) -> p g f", f=3)
                    for ns in range(4):
                        t = xF.tile([P, 512], F16, tag="F", name="F")
                        _unpack12(nc, upkA,
                                  wv[:, ns * 128:(ns + 1) * 128, :], t, 128)
                        F_sb[(1, dc, ns)] = t

            h_sb = {}  # (ti, u) -> [P, R] f32
            for u in range(4):
                for bank, tensors in ((0, (0, 1)), (1, (2,))):
                    ps = []
                    for ns in range(4):
                        p = psA.tile([P, 512], F32, name="psA")
                        for dc in range(8):
                            nc.tensor.matmul(
                                p[:], xT_sb[dc][:, u * P:(u + 1) * P],
                                F_sb[(bank, dc, ns)][:],
                                start=(dc == 0), stop=(dc == 7))
                        ps.append(p)
                    for ti in tensors:
                        tmp = tmpp.tile([P, NR], F32, tag="tmp", name="tmp")
                        for ns in range(4):
                            p3 = ps[ns][:].rearrange("p (n r) -> p n r", n=8)
                            w3 = fw_sb[ti][:, u * N:(u + 1) * N] \
                                [:, ns * 8:(ns + 1) * 8] \
                                .unsqueeze(2).broadcast_to([P, 8, R])
                            tv = tmp[:].rearrange("p (r n) -> p n r", r=R)[
                                :, ns * 8:(ns + 1) * 8, :]
                            nc.vector.tensor_tensor(out=tv, in0=p3, in1=w3, op=MULT)
                        h = hp.tile([P, R], F32, tag="h", name="h")
                        nc.vector.reduce_sum(
                            out=h[:],
                            in_=tmp[:].rearrange("p (r n) -> p r n", r=R),
                            axis=AXX)
                        h_sb[(ti, u)] = h

            for ti in range(3):
                hT = hT_pool.tile([R, TOK], F16, tag="hT", name="hT")
                for u in range(4):
                    tp = psH.tile([R, P], F32, name="psH")
                    nc.tensor.transpose(tp[:], h_sb[(ti, u)][:], ident32[:])
                    cp(hT[:, u * P:(u + 1) * P], tp[:])
                nc.sync.dma_start(out=hT_stack[ti * R:(ti + 1) * R, :], in_=hT[:])

        nc.gpsimd.collective_compute(
            "AllGather", mybir.AluOpType.bypass, replica_groups=GROUPS,
            ins=[hT_stack[:].opt()], outs=[hT_gath[:].opt()])

        # h2[ti]: [128, T] f16; rows 0-63 and 64-127 both = full hT for ti
        h2pool = ctx.enter_context(tc.tile_pool(name="h2", bufs=3))
        h2 = []
        gv = hT_gath[:].rearrange("(c x) t -> c x t", c=8)
        for ti in range(3):
            t = h2pool.tile([P, T], F16, name="h2")
            src = gv[:, ti * R:(ti + 1) * R, :].rearrange("c r t -> r c t")
            for half in range(2):
                nc.sync.dma_start(
                    out=t[half * R:(half + 1) * R, :]
                        .rearrange("r (c t) -> r c t", c=8),
                    in_=src)
            h2.append(t)

        # ---------------- Phase B: restore (nr-sharded) ----------------
        with tc.tile_pool(name="Rp", bufs=6) as Rp, \
             tc.tile_pool(name="wrp", bufs=2) as wrp, \
             tc.tile_pool(name="gp", bufs=4) as gp, \
             tc.tile_pool(name="ysb", bufs=4) as ysb, \
             tc.tile_pool(name="psB", bufs=4, space="PSUM") as psB:
            R_sb = {}  # (bank, ch) -> [P, D] f16
            for ch in range(2):
                t = Rp.tile([P, D], F16, tag="R", name="R")
                nc.sync.dma_start(out=t[:], in_=Rqk_d[ch * P:(ch + 1) * P, :])
                R_sb[(0, ch)] = t
            with tc.tile_pool(name="rpk", bufs=2) as rpk, \
                 tc.tile_pool(name="upkB", bufs=4) as upkB:
                for ch in range(2):
                    pt = rpk.tile([P, 768], U16, tag="Rvp", name="Rvp")
                    nc.sync.dma_start(out=pt[:],
                                      in_=Rvp_d[ch * P:(ch + 1) * P, :])
                    t = Rp.tile([P, D], F16, tag="R", name="R")
                    _unpack12(nc, upkB,
                              pt[:].rearrange("p (g f) -> p g f", f=3), t, 256)
                    R_sb[(1, ch)] = t
            for ti, bank in ((0, 0), (1, 0), (2, 1)):
                g = []
                for ch in range(2):
                    wr = wrp.tile([P, T], F16, tag="wr", name="wr")
                    for hh in range(2):
                        nn = 2 * ch + hh
                        nc.sync.dma_start(
                            out=wr[hh * R:(hh + 1) * R, :],
                            in_=rwT_d[ti][nn:nn + 1, :].broadcast_to([R, T]))
                    gt = gp.tile([P, T], F16, tag="g", name="g")
                    nc.vector.tensor_tensor(out=gt[:], in0=h2[ti][:], in1=wr[:],
                                            op=MULT)
                    g.append(gt)
                for tcn in range(8):
                    for dc in range(8):
                        ps = psB.tile([P, 512], F32, name="psB")
                        for ch in range(2):
                            nc.tensor.matmul(
                                ps[:],
                                R_sb[(bank, ch)][:, dc * P:(dc + 1) * P],
                                g[ch][:, tcn * 512:(tcn + 1) * 512],
                                start=(ch == 0), stop=(ch == 1))
                        y = ysb.tile([P, 512], F32, tag="y", name="y")
                        cp(y[:], ps[:])
                        nc.sync.dma_start(
                            out=yt_part[ti][dc * P:(dc + 1) * P,
                                            tcn * 512:(tcn + 1) * 512],
                            in_=y[:])

        for ti in range(3):
            nc.gpsimd.collective_compute(
                "ReduceScatter", ADD, replica_groups=GROUPS,
                ins=[yt_part[ti][:].opt()], outs=[yt_full[ti][:].opt()])

        # ---------------- Phase C: attention (2 heads x 2 batches) ----------
        qkv_pool = ctx.enter_context(tc.tile_pool(name="qkv", bufs=2))
        QT = qkv_pool.tile([P, T], F32, tag="QT", name="QT", bufs=1)
        KT = qkv_pool.tile([P, T], F32, tag="KT", name="KT", bufs=1)
        nc.sync.dma_start(out=QT[:], in_=yt_full[0][:])
        nc.sync.dma_start(out=KT[:], in_=yt_full[1][:])
        V_sb = []
        vsb = ctx.enter_context(tc.tile_pool(name="vsb", bufs=32))
        with tc.tile_pool(name="vload", bufs=1) as vload, \
             tc.tile_pool(name="psV", bufs=2, space="PSUM") as psV:
            Vt = vload.tile([P, T], F32, name="Vt")
            nc.sync.dma_start(out=Vt[:], in_=yt_full[2][:])
            for tt in range(32):
                tp = psV.tile([P, P], F32, name="psV")
                nc.tensor.transpose(tp[:], Vt[:, tt * P:(tt + 1) * P], ident32[:])
                v = vsb.tile([P, DL], F16, tag="V", name="V")
                cp(v[:], tp[:])
                V_sb.append(v)

        wot_pool = ctx.enter_context(tc.tile_pool(name="wot", bufs=1))
        WOT_sb = wot_pool.tile([P, D], F16, name="wot")
        with tc.tile_pool(name="wpk", bufs=1) as wpk, \
             tc.tile_pool(name="upkW", bufs=4) as upkW:
            pt = wpk.tile([P, 768], U16, tag="Wp", name="Wp")
            nc.sync.dma_start(out=pt[:], in_=WOTp_d)
            _unpack12(nc, upkW, pt[:].rearrange("p (g f) -> p g f", f=3),
                      WOT_sb, 256)

        with tc.tile_pool(name="expS", bufs=2) as Ep, \
             tc.tile_pool(name="expT", bufs=4) as Tp, \
             tc.tile_pool(name="aop", bufs=4) as Ap, \
             tc.tile_pool(name="osb", bufs=4) as Op, \
             tc.tile_pool(name="small", bufs=32) as smp, \
             tc.tile_pool(name="psS", bufs=4, space="PSUM") as psS, \
             tc.tile_pool(name="psT", bufs=2, space="PSUM") as psT, \
             tc.tile_pool(name="psAV", bufs=1, space="PSUM") as psAV, \
             tc.tile_pool(name="psWO", bufs=1, space="PSUM") as psWO:
            for b in range(2):
                for qt in range(16):
                    L = (qt + 1) * P
                    nb = (L + 511) // 512
                    q0 = b * S + qt * P
                    ao_pair = Ap.tile([P, DL], F16, tag="ao", name="ao")
                    for head in range(2):
                        qoff = head * DH
                        ps_s = []
                        mxs = []
                        for kb in range(nb):
                            Ls = min(512, L - kb * 512)
                            p = psS.tile([P, 512], F32, name="psS")
                            nc.tensor.matmul(
                                p[:, :Ls],
                                _r(QT[qoff:qoff + DH, q0:q0 + P]),
                                _r(KT[qoff:qoff + DH,
                                      b * S + kb * 512:b * S + kb * 512 + Ls]),
                                start=True, stop=True)
                            if kb == nb - 1:
                                nc.vector.scalar_tensor_tensor(
                                    out=p[:, Ls - P:Ls], in0=maskU[:],
                                    scalar=-1e30, in1=p[:, Ls - P:Ls],
                                    op0=MULT, op1=ADD)
                            mx = smp.tile([P, 1], F32, tag="mx", name="mx")
                            nc.vector.reduce_max(out=mx[:], in_=p[:, :Ls],
                                                 axis=AXX)
                            ps_s.append(p)
                            mxs.append(mx)
                        m = mxs[0]
                        for mx in mxs[1:]:
                            m2 = smp.tile([P, 1], F32, tag="mx", name="mx")
                            nc.vector.tensor_max(m2[:], m[:], mx[:])
                            m = m2
                        negm = smp.tile([P, 1], F32, tag="mx", name="mx")
                        nc.vector.tensor_scalar_mul(negm[:], m[:], -0.125)
                        expS = Ep.tile([P, S], F16, tag="e", name="e")
                        dens = []
                        for kb in range(nb):
                            Ls = min(512, L - kb * 512)
                            den = smp.tile([P, 1], F32, tag="mx", name="mx")
                            nc.scalar.activation(
                                expS[:, kb * 512:kb * 512 + Ls],
                                ps_s[kb][:, :Ls], EXP,
                                bias=negm[:], scale=0.125, accum_out=den[:])
                            dens.append(den)
                        dtot = dens[0]
                        for den in dens[1:]:
                            d2 = smp.tile([P, 1], F32, tag="mx", name="mx")
                            nc.vector.tensor_tensor(out=d2[:], in0=dtot[:],
                                                    in1=den[:], op=ADD)
                            dtot = d2
                        recip = smp.tile([P, 1], F32, tag="mx", name="mx")
                        nc.vector.reciprocal(recip[:], dtot[:])
                        att = psAV.tile([P, DH], F32, name="psAV")
                        for tb in range(qt + 1):
                            tp = psT.tile([P, P], F16, name="psT")
                            nc.tensor.transpose(
                                tp[:], expS[:, tb * P:(tb + 1) * P], ident16[:])
                            eT = Tp.tile([P, P], F16, tag="eT", name="eT")
                            cp(eT[:], tp[:])
                            nc.tensor.matmul(
                                att[:], eT[:],
                                V_sb[b * 16 + tb][:, qoff:qoff + DH],
                                start=(tb == 0), stop=(tb == qt))
                        nc.vector.tensor_scalar_mul(
                            ao_pair[:, qoff:qoff + DH], att[:], recip[:])
                    # W_O for this (b, qt) block
                    tp = psT.tile([P, P], F16, name="psT")
                    nc.tensor.transpose(tp[:], ao_pair[:], ident16[:])
                    aoT = Ap.tile([P, P], F16, tag="aoT", name="aoT")
                    cp(aoT[:], tp[:])
                    for dh in range(2):
                        ps = psWO.tile([P, 512], F32, name="psWO")
                        nc.tensor.matmul(
                            ps[:], aoT[:], WOT_sb[:, dh * 512:(dh + 1) * 512],
                            start=True, stop=True)
                        osb = Op.tile([P, 512], F32, tag="osb", name="osb")
                        cp(osb[:], ps[:])
                        nc.sync.dma_start(
                            out=out_part[q0:q0 + P, dh * 512:(dh + 1) * 512],
                            in_=osb[:])

        nc.gpsimd.collective_compute(
            "ReduceScatter", ADD, replica_groups=GROUPS,
            ins=[out_part[:].opt()], outs=[out_rs[:].opt()])

        # downcast the reduced output slice to 12-bit codes (fp16 rounded to
        # 12 bits, >>4) and pack 4 codes per 3 uint16 words for the wire
        with tc.tile_pool(name="fin", bufs=4) as fin, \
             tc.tile_pool(name="pk", bufs=10) as pk:
            for tt in range(4):
                f = fin.tile([P, D], F32, tag="f32", name="f32")
                nc.sync.dma_start(out=f[:], in_=out_rs[tt * P:(tt + 1) * P, :])
                f16t = fin.tile([P, D], F16, tag="f16", name="f16")
                nc.vector.tensor_copy(f16t[:], f[:])
                hu = f16t[:].bitcast(U16)
                # round-to-nearest-even at bit 4: c = (h + 8 + ((h>>4)&1)) >> 4
                lsb = pk.tile([P, D], U16, tag="w", name="lsb")
                nc.vector.tensor_scalar(out=lsb[:], in0=hu, scalar1=4,
                                        scalar2=1, op0=SHR, op1=AND)
                hr = pk.tile([P, D], U16, tag="w", name="hr")
                nc.vector.tensor_tensor(out=hr[:], in0=hu, in1=lsb[:], op=ADD)
                hr8 = pk.tile([P, D], U16, tag="w", name="hr8")
                nc.vector.tensor_scalar_add(hr8[:], hr[:], 8)
                c = pk.tile([P, D], U16, tag="w", name="c")
                nc.vector.tensor_scalar(out=c[:], in0=hr8[:], scalar1=4,
                                        scalar2=None, op0=SHR)
                cg = c[:].rearrange("p (g f) -> p g f", f=4)
                c0, c1, c2, c3 = (cg[:, :, i] for i in range(4))
                w = fin.tile([P, 3 * D // 4], U16, tag="wout", name="wout")
                wg = w[:].rearrange("p (g f) -> p g f", f=3)
                G = D // 4
                # w0 = c0 | ((c1 & 0xF) << 12)
                a = pk.tile([P, G], U16, tag="s", name="a")
                nc.vector.tensor_scalar(out=a[:], in0=c1, scalar1=0xF,
                                        scalar2=12, op0=AND, op1=SHL)
                nc.vector.tensor_tensor(out=wg[:, :, 0], in0=c0, in1=a[:], op=ORR)
                # w1 = (c1 >> 4) | ((c2 & 0xFF) << 8)
                b = pk.tile([P, G], U16, tag="s", name="b")
                nc.vector.tensor_scalar(out=b[:], in0=c1, scalar1=4,
                                        scalar2=None, op0=SHR)
                b2 = pk.tile([P, G], U16, tag="s", name="b2")
                nc.vector.tensor_scalar(out=b2[:], in0=c2, scalar1=0xFF,
                                        scalar2=8, op0=AND, op1=SHL)
                nc.vector.tensor_tensor(out=wg[:, :, 1], in0=b[:], in1=b2[:], op=ORR)
                # w2 = (c2 >> 8) | (c3 << 4)
                d1 = pk.tile([P, G], U16, tag="s", name="d1")
                nc.vector.tensor_scalar(out=d1[:], in0=c2, scalar1=8,
                                        scalar2=None, op0=SHR)
                d2 = pk.tile([P, G], U16, tag="s", name="d2")
                nc.vector.tensor_scalar(out=d2[:], in0=c3, scalar1=4,
                                        scalar2=None, op0=SHL)
                nc.vector.tensor_tensor(out=wg[:, :, 2], in0=d1[:], in1=d2[:], op=ORR)
                nc.sync.dma_start(out=out_d[tt * P:(tt + 1) * P, :], in_=w[:])
    nc.finalize()
    return nc


def kernel(x, fqk_weights_Q, fqk_weights_K, fv_weights,
           rqk_weights_Q, rqk_weights_K, rv_weights,
           f_qk, f_v, r_qk, r_v, W_O):
    f16 = np.float16
    x2 = np.asarray(x, np.float32).reshape(T, D)
    F_qk = np.asarray(f_qk, np.float32).transpose(1, 0, 2).reshape(D, NR)
    F_v = np.asarray(f_v, np.float32).transpose(1, 0, 2).reshape(D, NR)
    R_qk = np.asarray(r_qk, np.float32).reshape(NR, D).astype(f16)
    R_v = np.asarray(r_v, np.float32).reshape(NR, D).astype(f16)
    W_OT = np.asarray(W_O, np.float32).T.astype(f16)

    fw = [np.asarray(a, np.float32).reshape(T, N).astype(f16) for a in
          (fqk_weights_Q, fqk_weights_K, fv_weights)]
    rwT = [np.ascontiguousarray(
               np.asarray(a, np.float32).reshape(T, N).T).astype(f16)
           for a in (rqk_weights_Q, rqk_weights_K, rv_weights)]

    def _pack12h(a16):
        # fp16 -> 12-bit codes (RTN at bit 4), 4 codes per 3 uint16 words
        h = np.ascontiguousarray(a16).view(np.uint16).astype(np.uint32)
        cc = (h + 8 + ((h >> 4) & 1)) >> 4
        cc = cc.reshape(a16.shape[0], -1, 4)
        w0 = cc[..., 0] | ((cc[..., 1] & 0xF) << 12)
        w1 = (cc[..., 1] >> 4) | ((cc[..., 2] & 0xFF) << 8)
        w2 = (cc[..., 2] >> 8) | (cc[..., 3] << 4)
        return np.stack([w0, w1, w2], -1).astype(np.uint16) \
            .reshape(a16.shape[0], -1)

    def _u16(a16, rows):
        return np.ascontiguousarray(a16).view(np.uint16).reshape(rows, 4096)

    in_maps = []
    gblob = np.empty((8 * 400, 4096), np.uint16)
    _CACHED["global_blob"] = gblob
    for c in range(8):
        blob = gblob[400 * c:400 * (c + 1)]
        blob[0:128] = _u16(np.ascontiguousarray(
            x2[c * TOK:(c + 1) * TOK, :].T).astype(f16), 128)
        blob[128:192] = _u16(F_qk[c * P:(c + 1) * P, :].astype(f16), 64)
        blob[192:240] = _pack12h(
            F_v[c * P:(c + 1) * P, :].astype(f16)).reshape(48, 4096)
        for i, arr in enumerate(fw):
            blob[240 + 4 * i:244 + 4 * i] = \
                _u16(arr[c * TOK:(c + 1) * TOK, :], 4)
        for i, arr in enumerate(rwT):
            blob[252 + 4 * i:256 + 4 * i] = \
                _u16(arr[c * NL:(c + 1) * NL, :], 4)
        blob[264:328] = _u16(R_qk[c * KRL:(c + 1) * KRL, :], 64)
        blob[328:376] = _pack12h(
            R_v[c * KRL:(c + 1) * KRL, :]).reshape(48, 4096)
        blob[376:400] = _pack12h(
            W_OT[c * DL:(c + 1) * DL, :]).reshape(24, 4096)
        in_maps.append({"blob": blob})

    if "nc" not in _CACHED:
        _CACHED["nc"] = build()
    res = run_bass_kernel_spmd(_CACHED["nc"], in_maps, list(range(8)))
    out = np.empty((T, D), np.float32)
    for c in range(8):
        w = res.results[c]["outp"].reshape(TOK, D // 4, 3).astype(np.uint32)
        w0, w1, w2 = w[..., 0], w[..., 1], w[..., 2]
        codes = np.stack([w0 & 0xFFF,
                          ((w0 >> 12) | (w1 << 4)) & 0xFFF,
                          ((w1 >> 8) | (w2 << 8)) & 0xFFF,
                          w2 >> 4], axis=-1).reshape(TOK, D)
        h = (codes << 4).astype(np.uint16).view(np.float16)
        out[c * TOK:(c + 1) * TOK, :] = h.astype(np.float32)
    return out.reshape(B, S, D)


if __name__ == "__main__":
    d = np.load("/tmp/inputs.npz")
    out = kernel(**{k: d[k] for k in d.files})
    ref = np.load("/tmp/ref_out.npy")
    rel = np.linalg.norm(out - ref) / np.linalg.norm(ref)
    print("rel fro err:", rel)
